# revision 13
# baseline (speedup 1.0000x reference)
"""Causal self-attention kernel for 8 TRN2 NeuronCores (Bass/Tile).

Self-contained: accepts FULL inputs (x, w_attn, b_attn, w_proj, b_proj),
shards internally (core = 2*batch + head_group, Megatron-style head split),
runs one SPMD Bass program on cores 0-7, and gathers/sums partial outputs
on the host.

v2: bf16 datapath, V fused into the QKV phase, post-exp multiplicative
causal mask, chunk-pipelined emission (QKV of chunk c+1 and projection of
chunk c-1 interleaved into attention of chunk c) to keep the PE busy.
"""
import sys, types
from contextlib import ExitStack

import numpy as np
import ml_dtypes

# ---- NTFF profile hook ---------------------------------------------------
if "antenv.axon_hooks" not in sys.modules:
    mod = types.ModuleType("antenv.axon_hooks")
    _hook = [None]
    mod.set_axon_ntff_profile_hook = lambda h: _hook.__setitem__(0, h)
    mod.get_axon_ntff_profile_hook = lambda: _hook[0]
    sys.modules["antenv.axon_hooks"] = mod
    import antenv

    antenv.axon_hooks = mod
    try:
        from trn_agent_boot.trn_boot import _ntff_profile_via_ctypes

        mod.set_axon_ntff_profile_hook(
            _ntff_profile_via_ctypes("/opt/axon/libaxon_pjrt.so")
        )
    except Exception:
        pass

import concourse.bass as bass  # noqa: E402
import concourse.tile as tile  # noqa: E402
from concourse import mybir  # noqa: E402

# ---- split tail-drain waits ----------------------------------------------
if not getattr(tile.TileContext, "_drain_patched", False):

    def _patched_drain_and_barrier(self, tick_clock, wait_clock):
        drain_inst = self.nc.sync.drain()
        wait_clock.add_sem_waits(
            drain_inst.ins, tile.ScopedClock({None: tick_clock.global_clock})
        )
        si = drain_inst.ins.sync_info
        waits = list(si.on_wait) if si and si.on_wait else []
        if waits:
            si.on_wait = []
            for w in waits:
                nop = self.nc.sync.nop(nofuse=True, hint="drain_split")
                nop.ins.sync_info = mybir.SyncInfo(on_wait=[w], on_update=[])
        self.nc.all_engine_barrier()
        popped = self.nc._tile_sem_poison_stack.pop()
        assert popped is self._sem_poison
        self.nc.clear_and_free_semaphores(list(self.sems.allocated().values()))
        self.nc.all_engine_barrier()

    tile.TileContext._drain_and_barrier = _patched_drain_and_barrier
    tile.TileContext._drain_patched = True


_split_ctr = [0]


def split_multi_waits(nc):
    """Walrus in this image only accepts one sync-wait per instruction.

    For every instruction carrying N>1 waits, hoist the first N-1 waits onto
    fresh single-wait NoOps inserted just before it on the same engine queue
    (engine queues execute in order, so semantics are preserved).
    """
    for f in nc.m.functions:
        for bb in f.blocks:
            insts = bb.instructions
            out = []
            for ins in insts:
                si = ins.sync_info
                waits = list(si.on_wait) if si and si.on_wait else []
                if len(waits) > 1:
                    for w in waits[:-1]:
                        _split_ctr[0] += 1
                        nop = mybir.InstNoOp(
                            name=f"waitsplit_{_split_ctr[0]}", ins=[], outs=[]
                        )
                        nop.engine = ins.engine
                        nop.sync_info = mybir.SyncInfo(on_wait=[w], on_update=[])
                        out.append(nop)
                    si.on_wait = [waits[-1]]
                out.append(ins)
            if len(out) != len(insts):
                insts[:] = out

from concourse.bass_utils import run_bass_kernel_spmd  # noqa: E402


F32 = mybir.dt.float32
BF16 = mybir.dt.bfloat16
AF = mybir.ActivationFunctionType
ALU = mybir.AluOpType

B, T, C, H = 4, 2048, 1024, 16
HD = C // H           # head dim = 64
HG = H // 2           # heads per core = 8
DG = HG * HD          # 512 d-channels per core
KT_TILES = C // 128   # 8 contraction tiles over C
NCH = T // 512        # 4 token chunks of 512
VW = 4 * 130          # Vone width: per pair [V_even(64)|1|1|V_odd(64)]


def build_attn_nc():
    nc = bass.Bass()
    xT = nc.declare_dram_parameter("xT", [C, T], BF16, isOutput=False)
    wqkv = nc.declare_dram_parameter("wqkv", [C, 3 * DG], BF16, isOutput=False)
    bqkv = nc.declare_dram_parameter("bqkv", [2 * DG, 1], F32, isOutput=False)
    wproj = nc.declare_dram_parameter("wproj", [DG, C], BF16, isOutput=False)
    tri01 = nc.declare_dram_parameter("tri01", [128, 128], BF16, isOutput=False)
    ones = nc.declare_dram_parameter("ones", [128, 8], BF16, isOutput=False)
    y = nc.declare_dram_parameter("y", [T, C], F32, isOutput=True)
    dscratch = nc.dram_tensor("dscratch", [16, 1024], F32)   # raw denoms
    rscratch = nc.dram_tensor("rscratch", [16, 1024], F32)   # reciprocals

    dmae = [nc.sync, nc.scalar]  # the two HWDGE queues (scalar: prologue only)

    with tile.TileContext(nc) as tc, ExitStack() as ctx:
        const = ctx.enter_context(tc.tile_pool(name="const", bufs=1))
        big = ctx.enter_context(tc.tile_pool(name="big", bufs=1))
        xt_pool = ctx.enter_context(tc.tile_pool(name="xt", bufs=16))
        p_pool = ctx.enter_context(tc.tile_pool(name="p", bufs=6))
        r_pool = ctx.enter_context(tc.tile_pool(name="r", bufs=2))
        ostage = ctx.enter_context(tc.tile_pool(name="ost", bufs=4))
        mm_ps = ctx.enter_context(tc.tile_pool(name="mm_ps", bufs=2,
                                               space="PSUM"))
        s_ps = ctx.enter_context(tc.tile_pool(name="s_ps", bufs=2,
                                              space="PSUM"))
        o_ps = ctx.enter_context(tc.tile_pool(name="o_ps", bufs=2,
                                              space="PSUM"))

        # ---------------- constants / weights ----------------------------
        # DMA priority: x tiles of chunk 0 + Q weights first so the PE can
        # start, then K, V weights; wproj is deferred to the chunk-0 body.
        wq_sb, wk_sb, wv_sb = [], [], []
        for part, lst in (("wq", wq_sb), ("wk", wk_sb), ("wv", wv_sb)):
            for k in range(KT_TILES):
                lst.append(const.tile([128, DG], BF16, tag=f"{part}{k}",
                                      name=f"{part}{k}"))
        w_parts = {0: wq_sb, 1: wk_sb, 2: wv_sb}

        def stage_w():
            for qk in range(3):
                for k in range(KT_TILES):
                    dmae[k % 2].dma_start(
                        out=w_parts[qk][k][:],
                        in_=wqkv[k * 128:(k + 1) * 128,
                                 qk * DG:(qk + 1) * DG],
                    )

        tri_sb = const.tile([128, 128], BF16, tag="tri")
        bq_sb, bk_sb = [], []
        for m in range(4):
            bq_sb.append(const.tile([128, 1], F32, tag=f"bq{m}",
                                    name=f"bq{m}"))
            bk_sb.append(const.tile([128, 1], F32, tag=f"bk{m}",
                                    name=f"bk{m}"))

        def stage_small():
            nc.sync.dma_start(out=tri_sb[:], in_=tri01[:])
            for m in range(4):
                nc.sync.dma_start(out=bq_sb[m][:],
                                  in_=bqkv[m * 128:(m + 1) * 128, :])
                nc.sync.dma_start(
                    out=bk_sb[m][:],
                    in_=bqkv[DG + m * 128:DG + (m + 1) * 128, :],
                )

        wp_sb = [const.tile([128, C], BF16, tag=f"wp{kk}", name=f"wp{kk}")
                 for kk in range(4)]

        def stage_wp():
            for kk in range(4):
                dmae[kk % 2].dma_start(
                    out=wp_sb[kk][:], in_=wproj[kk * 128:(kk + 1) * 128, :]
                )

        # persistent per-chunk tensors (all bf16)
        QT = {(m, c): big.tile([128, 512], BF16, tag=f"qt{m}_{c}",
                               name=f"qt{m}_{c}")
              for m in range(4) for c in range(NCH)}
        KTc = {(m, c): big.tile([128, 512], BF16, tag=f"kt{m}_{c}",
                                name=f"kt{m}_{c}")
               for m in range(4) for c in range(NCH)}
        Vone = [big.tile([128, VW], BF16, tag=f"v{tt}", name=f"v{tt}")
                for tt in range(16)]
        OTc = {(m, c): big.tile([128, 512], BF16, tag=f"ot{m}_{c}",
                                name=f"ot{m}_{c}")
               for m in range(4) for c in range(NCH)}

        # ---------------- emission helpers -------------------------------
        def stage_xt(nch, xt_sb):
            ncs = nch * 512
            for k in range(KT_TILES):
                xt_t = xt_pool.tile([128, 512], BF16, tag="xt",
                                    name=f"xt{nch}_{k}")
                nc.sync.dma_start(
                    out=xt_t[:],
                    in_=xT[k * 128:(k + 1) * 128, ncs:ncs + 512],
                )
                xt_sb.append(xt_t)

        def emit_qkv_group(nch, gi, xt_sb):
            """One QKV matmul group for chunk `nch`.

            gi 0..3: Q for pair m=gi; gi 4..7: K for m=gi-4;
            gi 8..11: V for token tile 4*nch + (gi-8).
            """
            if gi < 8:  # Q or K
                qk, m = (0, gi) if gi < 4 else (1, gi - 4)
                wsrc = wq_sb if qk == 0 else wk_sb
                ps = mm_ps.tile([128, 512], F32, tag="mm")
                for k in range(KT_TILES):
                    nc.tensor.matmul(
                        ps[:],
                        lhsT=wsrc[k][:, m * 128:(m + 1) * 128],
                        rhs=xt_sb[k][:],
                        start=(k == 0), stop=(k == KT_TILES - 1),
                    )
                dst = QT[m, nch] if qk == 0 else KTc[m, nch]
                bias = bq_sb[m] if qk == 0 else bk_sb[m]
                nc.vector.tensor_scalar_add(dst[:], ps[:], bias[:])
            else:  # V token tile
                tl = gi - 8
                tt = 4 * nch + tl
                ps = mm_ps.tile([128, 512], F32, tag="mm")
                for k in range(KT_TILES):
                    nc.tensor.matmul(
                        ps[:],
                        lhsT=xt_sb[k][:, tl * 128:(tl + 1) * 128],
                        rhs=wv_sb[k][:],
                        start=(k == 0), stop=(k == KT_TILES - 1),
                    )
                # Vone pair block m (130 cols): [V_even(64) | 1 | V_odd(64) | 1]
                vt = Vone[tt][:]
                psv = ps[:]
                # even heads: ps cols 128m..128m+64 -> Vone cols 130m..+64
                dst_e = bass.AP(tensor=vt.tensor, offset=vt.offset,
                                ap=[list(vt.ap[0]), [130, 4], [1, 64]])
                src_e = bass.AP(tensor=psv.tensor, offset=psv.offset,
                                ap=[list(psv.ap[0]), [128, 4], [1, 64]])
                nc.vector.tensor_copy(dst_e, src_e)
                # odd heads: ps cols 128m+64..128m+128 -> Vone cols 130m+65..
                dst_o = bass.AP(tensor=vt.tensor, offset=vt.offset + 65,
                                ap=[list(vt.ap[0]), [130, 4], [1, 64]])
                src_o = bass.AP(tensor=psv.tensor, offset=psv.offset + 64,
                                ap=[list(psv.ap[0]), [128, 4], [1, 64]])
                nc.vector.tensor_copy(dst_o, src_o)
                # ones columns at 130m+64 and 130m+129
                dst_1 = bass.AP(tensor=vt.tensor, offset=vt.offset + 64,
                                ap=[list(vt.ap[0]), [130, 4], [65, 2]])
                nc.sync.dma_start(
                    out=dst_1,
                    in_=ones[:].rearrange("p (a b) -> p a b", a=4),
                )

        def emit_proj_group(c, gi):
            """Projection for token tile 4c+(gi//2), col half gi%2."""
            tl, ncol = gi // 2, gi % 2
            tt = 4 * c + tl
            ps = mm_ps.tile([128, 512], F32, tag="mm")
            for m in range(4):
                nc.tensor.matmul(
                    ps[:],
                    lhsT=OTc[m, c][:, tl * 128:(tl + 1) * 128],
                    rhs=wp_sb[m][:, ncol * 512:(ncol + 1) * 512],
                    start=(m == 0), stop=(m == 3),
                )
            st_ = ostage.tile([128, 512], F32, tag="ost")
            nc.vector.tensor_copy(st_[:], ps[:])
            nc.sync.dma_start(
                out=y[tt * 128:(tt + 1) * 128,
                      ncol * 512:(ncol + 1) * 512],
                in_=st_[:],
            )

        def emit_attn_pair(c, m, fill_step):
            """Attention for head pair m (heads 2m, 2m+1) on chunk c."""
            npair = 2 * c + 2
            o_acc = [o_ps.tile([128, 512], F32, tag="o",
                               name=f"oacc{c}_{m}_{s}") for s in range(2)]
            jmax = 4 * c + 4
            for t in range(npair):
                j0, j1 = 2 * t, 2 * t + 1
                offs = [128 * (j % 4) if j // 4 == c else 0 for j in (j0, j1)]
                off0 = offs[0]
                s_ts, p_ts = [], []
                for s in range(2):
                    s_t = s_ps.tile([128, 1024], F32, tag="s",
                                    name=f"s{c}_{m}_{t}_{s}")
                    for ji, j in enumerate((j0, j1)):
                        off = offs[ji]
                        jc, jcol = j // 4, 128 * (j % 4)
                        nc.tensor.matmul(
                            s_t[:, ji * 512 + off:(ji + 1) * 512],
                            lhsT=KTc[m, jc][s * 64:(s + 1) * 64,
                                            jcol:jcol + 128],
                            rhs=QT[m, c][s * 64:(s + 1) * 64, off:512],
                            start=True, stop=True,
                        )
                    s_ts.append(s_t)
                for s in range(2):
                    p_t = p_pool.tile([128, 1024], BF16, tag="p",
                                      name=f"p{c}_{m}_{t}_{s}")
                    nc.scalar.activation(
                        p_t[:, off0:1024], s_ts[s][:, off0:1024], AF.Exp,
                    )
                    if t >= 2 * c:  # both j0, j1 are diagonal tiles
                        pt = p_t[:]
                        dst = bass.AP(tensor=pt.tensor,
                                      offset=pt.offset + off0,
                                      ap=[list(pt.ap[0]), [640, 2], [1, 128]])
                        trs = tri_sb[:]
                        src = bass.AP(tensor=trs.tensor, offset=trs.offset,
                                      ap=[list(trs.ap[0]), [0, 2], [1, 128]])
                        nc.vector.tensor_tensor(
                            out=dst, in0=dst, in1=src, op=ALU.mult,
                        )
                    p_ts.append(p_t)
                for s in range(2):
                    for ji, j in enumerate((j0, j1)):
                        off = offs[ji]
                        nc.tensor.matmul(
                            o_acc[s][0:65, off:512],
                            lhsT=Vone[j][:, 130 * m + s * 65:
                                         130 * m + s * 65 + 65],
                            rhs=p_ts[s][:, ji * 512 + off:(ji + 1) * 512],
                            start=(j == 0), stop=(j == jmax - 1),
                        )
                fill_step()  # keep the PE busy while ACT runs exp
            # evict unnormalized O^T (both heads) + denominator row
            row = 4 * c + m
            nc.vector.tensor_copy(OTc[m, c][0:64, :], o_acc[0][0:64, :])
            nc.vector.tensor_copy(OTc[m, c][64:128, :], o_acc[1][0:64, :])
            drow = r_pool.tile([1, 1024], F32, tag="drow", bufs=2)
            nc.vector.tensor_copy(drow[:, 0:512], o_acc[0][64:65, :])
            nc.vector.tensor_copy(drow[:, 512:1024], o_acc[1][64:65, :])
            nc.sync.dma_start(out=dscratch[row:row + 1, :], in_=drow[:])
            emit_pair_norm(c, m)

        def emit_pair_norm(c, m):
            """Normalize OTc[m, c] *= 1/denominator (per-pair pipelining).

            Denominators sit along the free dim of one partition; bounce
            through DRAM to transpose/broadcast them across partitions.
            All DMAs ride the in-order sync queue, so RAW through DRAM holds.
            """
            row = 4 * c + m
            dall = dscratch[:].rearrange("a b -> (a b)")
            rall = rscratch[:].rearrange("a b -> (a b)")
            qofs = row * 1024
            rc8 = r_pool.tile([128, 8], F32, tag="rc")
            nc.sync.dma_start(
                out=rc8[:],
                in_=bass.AP(tensor=dall.tensor, offset=dall.offset + qofs,
                            ap=[[8, 128], [1, 8]]),
            )
            rr8 = r_pool.tile([128, 8], F32, tag="rr")
            nc.vector.reciprocal(rr8[:], rc8[:])
            nc.sync.dma_start(
                out=bass.AP(tensor=rall.tensor, offset=rall.offset + qofs,
                            ap=[[8, 128], [1, 8]]),
                in_=rr8[:],
            )
            rb = r_pool.tile([128, 1024], F32, tag="rb", bufs=2,
                             name=f"rb{row}")
            rsrc = rscratch[row:row + 1, :]
            nc.sync.dma_start(
                out=rb[:],
                in_=bass.AP(tensor=rsrc.tensor, offset=rsrc.offset,
                            ap=[[0, 128], [1, 1024]]),
            )
            sl0 = OTc[m, c][0:64, :]
            nc.vector.tensor_tensor(out=sl0, in0=sl0,
                                    in1=rb[0:64, 0:512], op=ALU.mult)
            sl1 = OTc[m, c][64:128, :]
            nc.vector.tensor_tensor(out=sl1, in0=sl1,
                                    in1=rb[64:128, 512:1024],
                                    op=ALU.mult)

        # ---------------- schedule ---------------------------------------
        # prologue: x(0) + weights staged in use-order; Q0/K0/V* computed
        # up-front, the rest of chunk 0's QKV dribbles in as fill
        xt_chunks = {0: []}
        stage_xt(0, xt_chunks[0])
        stage_w()
        stage_small()
        for gi in (0, 4, 8, 9, 10, 11):    # Q0, K0, V0..V3
            emit_qkv_group(0, gi, xt_chunks[0])
        stage_wp()

        pending = [("qkv", 0, gi) for gi in (1, 5, 2, 6, 3, 7)]

        for c in range(NCH):
            if c + 1 < NCH:
                xt_chunks[c + 1] = []
                stage_xt(c + 1, xt_chunks[c + 1])
            # fill work: rest of QKV, then projection of c-1, paced evenly
            # across this chunk's attention t-loops to cover exp latency
            fill = list(pending)
            pending = []
            if c + 1 < NCH:
                fill += [("qkv", c + 1, gi) for gi in range(12)]
            if c >= 1:
                fill += [("proj", c - 1, gi) for gi in range(8)]
            fi = [0]
            slots = 4 * (2 * c + 2)
            pace = len(fill) / slots
            credit = [0.0]

            def fill_step():
                credit[0] += pace
                while fi[0] < len(fill) and credit[0] >= 1.0:
                    kind, cc, gi = fill[fi[0]]
                    if kind == "qkv":
                        emit_qkv_group(cc, gi, xt_chunks[cc])
                    else:
                        emit_proj_group(cc, gi)
                    fi[0] += 1
                    credit[0] -= 1.0

            for m in range(4):
                emit_attn_pair(c, m, fill_step)
            while fi[0] < len(fill):
                kind, cc, gi = fill[fi[0]]
                if kind == "qkv":
                    emit_qkv_group(cc, gi, xt_chunks[cc])
                else:
                    emit_proj_group(cc, gi)
                fi[0] += 1

        # epilogue: projection for the last chunk
        for gi in range(8):
            emit_proj_group(NCH - 1, gi)

    split_multi_waits(nc)
    return nc


def make_tri01() -> np.ndarray:
    k = np.arange(128)[:, None]
    q = np.arange(128)[None, :]
    return np.where(q >= k, 1.0, 0.0).astype(ml_dtypes.bfloat16)


def make_in_maps(x, w_attn, b_attn, w_proj):
    scale = np.float32(1.0 / np.sqrt(HD))
    tri01 = make_tri01()
    bf = ml_dtypes.bfloat16
    in_maps = []
    for core in range(8):
        b, g = divmod(core, 2)
        cs = g * DG
        wq = w_attn[:, cs:cs + DG] * scale
        wk = w_attn[:, C + cs:C + cs + DG]
        wv = w_attn[:, 2 * C + cs:2 * C + cs + DG]
        bq = b_attn[cs:cs + DG] * scale
        bk = b_attn[C + cs:C + cs + DG]
        in_maps.append({
            "xT": np.ascontiguousarray(x[b].T).astype(bf),
            "wqkv": np.ascontiguousarray(
                np.concatenate([wq, wk, wv], axis=1)).astype(bf),
            "bqkv": np.ascontiguousarray(
                np.concatenate([bq, bk]).reshape(-1, 1)).astype(np.float32),
            "wproj": np.ascontiguousarray(w_proj[cs:cs + DG, :]).astype(bf),
            "tri01": tri01,
            "ones": np.ones((128, 8), dtype=bf),
        })
    return in_maps


_NC_CACHE = []


def kernel_full(x, w_attn, b_attn, w_proj, b_proj, trace=False, nc=None):
    if nc is None:
        if not _NC_CACHE:
            _NC_CACHE.append(build_attn_nc())
        nc = _NC_CACHE[0]
    x = np.asarray(x, dtype=np.float32)
    w_attn = np.asarray(w_attn, dtype=np.float32)
    b_attn = np.asarray(b_attn, dtype=np.float32)
    w_proj = np.asarray(w_proj, dtype=np.float32)
    b_proj = np.asarray(b_proj, dtype=np.float32)
    in_maps = make_in_maps(x, w_attn, b_attn, w_proj)
    res = run_bass_kernel_spmd(nc, in_maps, list(range(8)), trace=trace)
    # exact host-side correction: softmax rows sum to 1, so the V bias
    # contributes bv @ w_proj to every output row
    bv_all = b_attn[2 * C:3 * C]
    const_row = (b_proj + bv_all @ w_proj).astype(np.float32)
    out = np.empty((B, T, C), dtype=np.float32)
    for b in range(B):
        out[b] = (res.results[2 * b]["y"] + res.results[2 * b + 1]["y"]
                  + const_row[None, :])
    return out, res


def kernel(x, w_attn, b_attn, w_proj, b_proj):
    out, _ = kernel_full(x, w_attn, b_attn, w_proj, b_proj, trace=False)
    return out
